# revision 30
# baseline (speedup 1.0000x reference)
"""Trainium2 Bass kernel for LongContextAttention (B=1, S=2048, H=16, D=128).

Strategy: permutations are pure data movement -> host-side numpy gathers.
Attention is head-parallel: 16 heads / 8 cores = 2 heads per core.

Per core, software-pipelined over 128 (h, q-chunk, k-tile) steps:
  - QK^T transposed: scoresT[k,q] = sum_d KT[d,k] QT[d,q]; stationary = KT tile
    (128x128), moving = QT chunk (128x512). PSUM banks rotate globally (g%6,
    6 banks) so the PE always runs ~6 k-tiles ahead of the exp.
  - exp on ScalarE (the bottleneck engine: 1 col/cycle @1.2GHz + ~158ns/instr)
    in groups of 2-4 k-tiles chosen per chunk (pattern depends on hq%3) so
    every group reads a contiguous, non-wrapping PSUM span.
  - PV with V as STATIONARY (16 weight loads per head instead of 256):
    psum_ctxT[d, q-chunk] accumulates over the 16 k-tiles, moving =
    pt[:, kt, qchunk] (512 cols). The PE stream is ordered per exp-group
    event: first the newly-unblocked QKs, then that group's PVs, which keeps
    the Scalar engine (critical path) fed with zero slack.
  - denominator: DVE pairwise tensor_add tree over kt (2x mode) -> s1[k, q]
    bf16 partials; the final 128-partition sum happens on the host.
  - ctxT PSUM -> SBUF copy on DVE, DMA out fp32. Host divides by the
    denominator and transposes (device output is ctx^T [h, d, q]).
"""

import numpy as np
import ml_dtypes

B, S, H, D = 1, 2048, 16, 128
BLOCK = 64
NCORES = 8
HPC = H // NCORES          # heads per core = 2
NT = S // 128              # 16 k-tiles
NQC = 4                    # q-chunks of 512
QC = 512
SCALE = 1.0 / float(np.sqrt(D))
NSTEP = HPC * NQC * NT     # 128 (h, qc, kt) steps
NHQ = HPC * NQC            # 8 (h, qc) chunks
NQKBUF = 6                 # PSUM banks rotating for QK output

# exp group sizes per chunk; chosen so each group's PSUM span (bank g%6)
# is contiguous and never wraps: pattern depends on (16*hq) % 6 cycling 0,4,2.
PATTERNS = {0: [4, 2, 4, 2, 4], 1: [2, 4, 2, 4, 2, 2], 2: [4, 2, 4, 2, 4]}
GRPS = []                  # (g0, n, hq, local_start)
GIDX = [None] * NSTEP      # step -> group index
for _hq in range(NHQ):
    _s = 0
    for _n in PATTERNS[_hq % 3]:
        _g0 = NT * _hq + _s
        assert (_g0 % NQKBUF) + _n <= NQKBUF
        GRPS.append((_g0, _n, _hq, _s))
        for _t in range(_n):
            GIDX[_g0 + _t] = len(GRPS) - 1
        _s += _n
    assert _s == NT
GRPS_END = [0] * NHQ       # number of groups up to and including hq
for _g0, _n, _hq, _ls in GRPS:
    GRPS_END[_hq] = max(GRPS_END[_hq], GRPS.index((_g0, _n, _hq, _ls)) + 1)

_CACHE = {}


def _build_bass():
    import concourse.bass as bass
    import concourse.mybir as mybir
    from contextlib import ExitStack

    f32 = mybir.dt.float32
    bf16 = mybir.dt.bfloat16

    nc = bass.Bass()
    kt_in = nc.declare_dram_parameter("kt_in", [HPC, D, S], bf16, isOutput=False)
    qt_in = nc.declare_dram_parameter("qt_in", [HPC, D, S], bf16, isOutput=False)
    va_in = nc.declare_dram_parameter("va_in", [HPC, 128, NT, D], bf16, isOutput=False)
    kq0_in = nc.declare_dram_parameter("kq0_in", [128, 1280], bf16, isOutput=False)
    ctx_out = nc.declare_dram_parameter("ctx_out", [HPC, D, S], f32, isOutput=True)
    s_out = nc.declare_dram_parameter("s_out", [HPC, NQC, 128, QC], f32, isOutput=True)

    ctx = ExitStack()
    with ctx:
        kt_sb = ctx.enter_context(nc.sbuf_tensor("kt_sb", [128, HPC, S], bf16))
        qt_sb = ctx.enter_context(nc.sbuf_tensor("qt_sb", [128, HPC, S], bf16))
        # prologue staging: kt0[:, 0:768] ++ qt0[:, 0:512] in one early DMA
        kq0_sb = ctx.enter_context(nc.sbuf_tensor("kq0_sb", [128, 1280], bf16))
        va_sb = ctx.enter_context(nc.sbuf_tensor("va_sb", [128, HPC, NT, D], bf16))
        pt_sb = [
            ctx.enter_context(nc.sbuf_tensor(f"pt_sb{i}", [128, NT, QC], bf16))
            for i in range(3)
        ]
        s2_sb = ctx.enter_context(nc.sbuf_tensor("s2_sb", [128, 2, QC], bf16))
        t1_sb = ctx.enter_context(nc.sbuf_tensor("t1_sb", [128, QC], bf16))
        s1_sb = ctx.enter_context(nc.sbuf_tensor("s1_sb", [128, 2, QC], f32))
        out_sb = ctx.enter_context(nc.sbuf_tensor("out_sb", [128, 2, QC], f32))
        warm_sb = ctx.enter_context(nc.sbuf_tensor("warm_sb", [128, 2], f32))

        psum_qk = ctx.enter_context(nc.psum_tensor("psum_qk", [128, NQKBUF * QC], f32))
        psum_cx = ctx.enter_context(nc.psum_tensor("psum_cx", [128, 2 * QC], f32))

        load_sems = [ctx.enter_context(nc.semaphore(f"load_sem{i}")) for i in range(7)]
        qk_sem = ctx.enter_context(nc.semaphore("qk_sem"))
        exp_sem = ctx.enter_context(nc.semaphore("exp_sem"))
        pv_sem = ctx.enter_context(nc.semaphore("pv_sem"))
        tree_sem = ctx.enter_context(nc.semaphore("tree_sem"))
        cp_sem = ctx.enter_context(nc.semaphore("cp_sem"))
        st_sem = ctx.enter_context(nc.semaphore("st_sem"))
        s1st_sem = ctx.enter_context(nc.semaphore("s1st_sem"))
        warm_sem = ctx.enter_context(nc.semaphore("warm_sem"))

        block = ctx.enter_context(nc.Block(no_gpsimd_drain=True))

        @block.sync
        def _(sync):
            # Full input loads. The urgent prologue slice (kq0) goes on the
            # gpsimd engine, which enters its block ~1us earlier than sync.
            # Each barrier gets its own semaphore: a shared counter races
            # when DMA queues progress unevenly.
            sync.dma_start(out=kt_sb[:, 0, :], in_=kt_in[0]).then_inc(load_sems[1], 16)
            sync.dma_start(out=qt_sb[:, 0, :], in_=qt_in[0]).then_inc(load_sems[3], 16)
            sync.dma_start(out=kt_sb[:, 1, :], in_=kt_in[1]).then_inc(load_sems[4], 16)
            sync.dma_start(out=qt_sb[:, 1, :], in_=qt_in[1]).then_inc(load_sems[6], 16)
            # Output ctx DMAs from here (sync is idle after the loads);
            # the s1 DMAs stay on gpsimd so the triggers run in parallel.
            for hq in range(NHQ):
                h, qc = divmod(hq, NQC)
                sync.wait_ge(cp_sem, hq + 1)
                sync.dma_start(
                    out=ctx_out[h][:, qc * QC : (qc + 1) * QC],
                    in_=out_sb[:, hq % 2, :],
                ).then_inc(st_sem, 16)
            sync.wait_ge(st_sem, 16 * NHQ)

        @block.tensor
        def _(tensor):
            def emit_qk(g):
                h, r = divmod(g, NQC * NT)
                qc, kt = divmod(r, NT)
                b = g % NQKBUF
                if g == 0:
                    tensor.wait_ge(load_sems[0], 16)   # kq0 prologue slice
                if g == NQKBUF:
                    tensor.wait_ge(load_sems[1], 16)   # kt0 full
                    tensor.wait_ge(load_sems[3], 16)   # qt0 full
                if g == NQC * NT:
                    tensor.wait_ge(load_sems[4], 16)   # kt1
                    tensor.wait_ge(load_sems[6], 16)   # qt1
                if g < NQKBUF:
                    lhs = kq0_sb[:, kt * 128 : (kt + 1) * 128]
                    rhs = kq0_sb[:, 768:1280]
                else:
                    lhs = kt_sb[:, h, kt * 128 : (kt + 1) * 128]
                    rhs = qt_sb[:, h, qc * QC : (qc + 1) * QC]
                tensor.matmul(
                    psum_qk[:, b * QC : (b + 1) * QC],
                    lhs,
                    rhs,
                    start=True,
                    stop=True,
                ).then_inc(qk_sem, 1)

            def emit_pv(g2):
                h2, r2 = divmod(g2, NQC * NT)
                qc2, kt2 = divmod(r2, NT)
                hq2 = g2 // NT
                cb = hq2 % 2
                pb2 = hq2 % 3
                if kt2 == 0:
                    tensor.wait_ge(load_sems[2] if h2 == 0 else load_sems[5], 16)
                    if hq2 >= 2:
                        tensor.wait_ge(cp_sem, hq2 - 1)  # psum_cx buf drained
                mm = tensor.matmul(
                    psum_cx[:, cb * QC : (cb + 1) * QC],
                    va_sb[:, h2, kt2, :],
                    pt_sb[pb2][:, kt2, :],
                    start=(kt2 == 0),
                    stop=(kt2 == NT - 1),
                )
                if kt2 == NT - 1:
                    mm.then_inc(pv_sem, 1)

            for g in range(NQKBUF):
                emit_qk(g)
            for gi, (g0, n, hq, ls) in enumerate(GRPS):
                tensor.wait_ge(exp_sem, gi + 1)
                for g in range(g0 + NQKBUF, min(g0 + NQKBUF + n, NSTEP)):
                    emit_qk(g)
                for g2 in range(g0, g0 + n):
                    emit_pv(g2)

        @block.scalar
        def _(scalar):
            import concourse.mybir as mybir_

            # Warm the Exp activation table while input DMAs run.
            scalar.wait_ge(warm_sem, 1)
            scalar.activation(
                out=warm_sb[:, 1:2],
                in_=warm_sb[:, 0:1],
                func=mybir_.ActivationFunctionType.Exp,
                scale=1.0,
            )
            for gi, (g0, n, hq, ls) in enumerate(GRPS):
                if ls == 0 and hq >= 3:
                    # pt buf reused from (hq-3): both PV and tree must be done
                    scalar.wait_ge(pv_sem, hq - 2)
                    scalar.wait_ge(tree_sem, hq - 2)
                scalar.wait_ge(qk_sem, g0 + n)
                b0 = g0 % NQKBUF
                scalar.activation(
                    out=pt_sb[hq % 3][:, ls : ls + n, :],
                    in_=psum_qk[:, b0 * QC : (b0 + n) * QC],
                    func=mybir_.ActivationFunctionType.Exp,
                    scale=SCALE,
                ).then_inc(exp_sem, 1)

        @block.vector
        def _(vector):
            # incremental kt-accumulation: after each exp group, fold its
            # k-tiles into s1 (fp32 accumulator) so the denominator finishes
            # right after the chunk's last exp instead of in a tail tree.
            for gi, (g0, n, hq, ls) in enumerate(GRPS):
                pb = hq % 3
                sb1 = hq % 2
                first = ls == 0
                last = ls + n == NT
                vector.wait_ge(exp_sem, gi + 1)
                if first and hq >= 2:
                    vector.wait_ge(s1st_sem, 16 * (hq - 1))
                with nc.allow_low_precision("bf16 partials; fp32 accumulator"):
                    if n == 4:
                        vector.tensor_add(
                            out=s2_sb[:, :, :],
                            in0=pt_sb[pb][:, ls : ls + 2, :],
                            in1=pt_sb[pb][:, ls + 2 : ls + 4, :],
                        )
                        if first:
                            mm = vector.tensor_add(
                                out=s1_sb[:, sb1, :], in0=s2_sb[:, 0, :], in1=s2_sb[:, 1, :]
                            )
                        else:
                            vector.tensor_add(
                                out=t1_sb[:, :], in0=s2_sb[:, 0, :], in1=s2_sb[:, 1, :]
                            )
                            mm = vector.tensor_add(
                                out=s1_sb[:, sb1, :], in0=s1_sb[:, sb1, :], in1=t1_sb[:, :]
                            )
                    else:
                        if first:
                            mm = vector.tensor_add(
                                out=s1_sb[:, sb1, :],
                                in0=pt_sb[pb][:, ls, :],
                                in1=pt_sb[pb][:, ls + 1, :],
                            )
                        else:
                            vector.tensor_add(
                                out=t1_sb[:, :],
                                in0=pt_sb[pb][:, ls, :],
                                in1=pt_sb[pb][:, ls + 1, :],
                            )
                            mm = vector.tensor_add(
                                out=s1_sb[:, sb1, :], in0=s1_sb[:, sb1, :], in1=t1_sb[:, :]
                            )
                if last:
                    mm.then_inc(tree_sem, 1)
                    # ctxT PSUM -> SBUF staging copy
                    vector.wait_ge(pv_sem, hq + 1)
                    if hq >= 2:
                        vector.wait_ge(st_sem, 16 * (hq - 1))
                    cb = hq % 2
                    vector.tensor_scalar_add(
                        out=out_sb[:, cb, :],
                        in0=psum_cx[:, cb * QC : (cb + 1) * QC],
                        scalar1=0.0,
                    ).then_inc(cp_sem, 1)

        @block.gpsimd
        def _(gpsimd):
            # Most urgent first: the prologue Q/K slice that QK(0..5) needs.
            gpsimd.dma_start(out=kq0_sb[:, :], in_=kq0_in[:, :]).then_inc(load_sems[0], 16)
            gpsimd.memset(warm_sb[:, 0:1], 0.0).then_inc(warm_sem, 1)
            gpsimd.dma_start(out=va_sb[:, 0, :, :], in_=va_in[0]).then_inc(load_sems[2], 16)
            gpsimd.dma_start(out=va_sb[:, 1, :, :], in_=va_in[1]).then_inc(load_sems[5], 16)
            for hq in range(NHQ):
                h, qc = divmod(hq, NQC)
                gpsimd.wait_ge(tree_sem, hq + 1)
                gpsimd.dma_start(
                    out=s_out[h, qc], in_=s1_sb[:, hq % 2, :]
                ).then_inc(s1st_sem, 16)
            gpsimd.wait_ge(s1st_sem, 16 * NHQ)

    return nc


def _perm_blocks(x, idx):
    xb = x.reshape(B, S // BLOCK, BLOCK, H, D)
    return xb[:, idx].reshape(B, S, H, D)


def kernel(**inputs):
    from concourse.bass_utils import run_bass_kernel_spmd

    q = np.asarray(inputs["query"], dtype=np.float32)
    k = np.asarray(inputs["key"], dtype=np.float32)
    v = np.asarray(inputs["value"], dtype=np.float32)
    hp = np.asarray(inputs["head_perm_idx"]).astype(np.int64)
    hd = np.asarray(inputs["head_deperm_idx"]).astype(np.int64)
    rp = np.asarray(inputs["new_row_perm_idx"]).astype(np.int64)
    cp = np.asarray(inputs["new_col_perm_idx"]).astype(np.int64)
    rd = np.asarray(inputs["new_row_deperm_idx"]).astype(np.int64)

    qp = _perm_blocks(q[:, :, hp], rp)[0]  # [S, H, D]
    kp = _perm_blocks(k[:, :, hp], cp)[0]
    vp = _perm_blocks(v[:, :, hp], cp)[0]

    bf = ml_dtypes.bfloat16
    qt = np.ascontiguousarray(qp.transpose(1, 2, 0)).astype(bf)  # [H, D, S]
    kt = np.ascontiguousarray(kp.transpose(1, 2, 0)).astype(bf)  # [H, D, S]
    # va[h, kp, kt, d] = V[h, kt*128 + kp, d]
    va = np.ascontiguousarray(
        vp.transpose(1, 0, 2).reshape(H, NT, 128, D).transpose(0, 2, 1, 3)
    ).astype(bf)

    if "nc" not in _CACHE:
        _CACHE["nc"] = _build_bass()
    nc = _CACHE["nc"]

    core_ids = list(range(NCORES))
    in_maps = [
        {
            "kt_in": np.ascontiguousarray(kt[c * HPC : (c + 1) * HPC]),
            "qt_in": np.ascontiguousarray(qt[c * HPC : (c + 1) * HPC]),
            "va_in": np.ascontiguousarray(va[c * HPC : (c + 1) * HPC]),
            "kq0_in": np.ascontiguousarray(
                np.concatenate(
                    [kt[c * HPC][:, 0:768], qt[c * HPC][:, 0:QC]], axis=1
                )
            ),
        }
        for c in core_ids
    ]
    res = run_bass_kernel_spmd(nc, in_maps, core_ids)
    _CACHE["last_result"] = res

    ctxT = np.concatenate(
        [res.results[c]["ctx_out"] for c in core_ids], axis=0
    )  # [H, D, S] fp32, unnormalized
    s1 = np.concatenate(
        [np.asarray(res.results[c]["s_out"], dtype=np.float32) for c in core_ids],
        axis=0,
    )  # [H, NQC, 128, QC]
    denom = s1.sum(axis=2).reshape(H, S)  # [H, S]
    ctxT = ctxT / denom[:, None, :]
    ctx = np.ascontiguousarray(ctxT.transpose(2, 0, 1))[None]  # [1, S, H, D]
    ctx = _perm_blocks(ctx, rd)
    out = ctx[:, :, hd]
    return np.ascontiguousarray(out, dtype=np.float32)


# revision 32
# speedup vs baseline: 1.1202x; 1.1202x over previous
"""Trainium2 Bass kernel for LongContextAttention (B=1, S=2048, H=16, D=128).

Strategy: permutations are pure data movement -> host-side numpy gathers.
Attention is head-parallel: 16 heads / 8 cores = 2 heads per core.

Per core, software-pipelined over 128 (h, q-chunk, k-tile) steps:
  - QK^T transposed: scoresT[k,q] = sum_d KT[d,k] QT[d,q]; stationary = KT tile
    (128x128), moving = QT chunk (128x512). PSUM banks rotate globally (g%6,
    6 banks) so the PE always runs ~6 k-tiles ahead of the exp.
  - exp on ScalarE (the bottleneck engine: 1 col/cycle @1.2GHz + ~158ns/instr)
    in groups of 2-4 k-tiles chosen per chunk (pattern depends on hq%3) so
    every group reads a contiguous, non-wrapping PSUM span.
  - PV with V as STATIONARY (16 weight loads per head instead of 256):
    psum_ctxT[d, q-chunk] accumulates over the 16 k-tiles, moving =
    pt[:, kt, qchunk] (512 cols). The PE stream is ordered per exp-group
    event: first the newly-unblocked QKs, then that group's PVs, which keeps
    the Scalar engine (critical path) fed with zero slack.
  - denominator: DVE pairwise tensor_add tree over kt (2x mode) -> s1[k, q]
    bf16 partials; the final 128-partition sum happens on the host.
  - ctxT PSUM -> SBUF copy on DVE, DMA out fp32. Host divides by the
    denominator and transposes (device output is ctx^T [h, d, q]).
"""

import numpy as np
import ml_dtypes

B, S, H, D = 1, 2048, 16, 128
BLOCK = 64
NCORES = 8
HPC = H // NCORES          # heads per core = 2
NT = S // 128              # 16 k-tiles
NQC = 4                    # q-chunks of 512
QC = 512
SCALE = 1.0 / float(np.sqrt(D))
NSTEP = HPC * NQC * NT     # 128 (h, qc, kt) steps
NHQ = HPC * NQC            # 8 (h, qc) chunks
NQKBUF = 6                 # PSUM banks rotating for QK output

# exp group sizes per chunk; chosen so each group's PSUM span (bank g%6)
# is contiguous and never wraps: pattern depends on (16*hq) % 6 cycling 0,4,2.
PATTERNS = {0: [3, 3, 3, 3, 3, 1], 1: [2, 3, 3, 3, 3, 2], 2: [4, 3, 3, 3, 3]}
GRPS = []                  # (g0, n, hq, local_start)
GIDX = [None] * NSTEP      # step -> group index
for _hq in range(NHQ):
    _s = 0
    for _n in PATTERNS[_hq % 3]:
        _g0 = NT * _hq + _s
        assert (_g0 % NQKBUF) + _n <= NQKBUF
        GRPS.append((_g0, _n, _hq, _s))
        for _t in range(_n):
            GIDX[_g0 + _t] = len(GRPS) - 1
        _s += _n
    assert _s == NT
GRPS_END = [0] * NHQ       # number of groups up to and including hq
for _g0, _n, _hq, _ls in GRPS:
    GRPS_END[_hq] = max(GRPS_END[_hq], GRPS.index((_g0, _n, _hq, _ls)) + 1)

_CACHE = {}


def _build_bass():
    import concourse.bass as bass
    import concourse.mybir as mybir
    from contextlib import ExitStack

    f32 = mybir.dt.float32
    bf16 = mybir.dt.bfloat16

    nc = bass.Bass()
    kt_in = nc.declare_dram_parameter("kt_in", [HPC, D, S], bf16, isOutput=False)
    qt_in = nc.declare_dram_parameter("qt_in", [HPC, D, S], bf16, isOutput=False)
    va_in = nc.declare_dram_parameter("va_in", [HPC, 128, NT, D], bf16, isOutput=False)
    kq0_in = nc.declare_dram_parameter("kq0_in", [128, 1280], bf16, isOutput=False)
    ctx_out = nc.declare_dram_parameter("ctx_out", [HPC, D, S], f32, isOutput=True)
    s_out = nc.declare_dram_parameter("s_out", [HPC, NQC, 128, QC], f32, isOutput=True)

    ctx = ExitStack()
    with ctx:
        kt_sb = ctx.enter_context(nc.sbuf_tensor("kt_sb", [128, HPC, S], bf16))
        qt_sb = ctx.enter_context(nc.sbuf_tensor("qt_sb", [128, HPC, S], bf16))
        # prologue staging: kt0[:, 0:768] ++ qt0[:, 0:512] in one early DMA
        kq0_sb = ctx.enter_context(nc.sbuf_tensor("kq0_sb", [128, 1280], bf16))
        va_sb = ctx.enter_context(nc.sbuf_tensor("va_sb", [128, HPC, NT, D], bf16))
        pt_sb = [
            ctx.enter_context(nc.sbuf_tensor(f"pt_sb{i}", [128, NT, QC], bf16))
            for i in range(3)
        ]
        s2_sb = ctx.enter_context(nc.sbuf_tensor("s2_sb", [128, 2, QC], bf16))
        t1_sb = ctx.enter_context(nc.sbuf_tensor("t1_sb", [128, QC], bf16))
        s1_sb = ctx.enter_context(nc.sbuf_tensor("s1_sb", [128, 2, QC], f32))
        out_sb = ctx.enter_context(nc.sbuf_tensor("out_sb", [128, 2, QC], f32))
        warm_sb = ctx.enter_context(nc.sbuf_tensor("warm_sb", [128, 2], f32))

        psum_qk = ctx.enter_context(nc.psum_tensor("psum_qk", [128, NQKBUF * QC], f32))
        psum_cx = ctx.enter_context(nc.psum_tensor("psum_cx", [128, 2 * QC], f32))

        load_sems = [ctx.enter_context(nc.semaphore(f"load_sem{i}")) for i in range(7)]
        qk_sem = ctx.enter_context(nc.semaphore("qk_sem"))
        exp_sem = ctx.enter_context(nc.semaphore("exp_sem"))
        pv_sem = ctx.enter_context(nc.semaphore("pv_sem"))
        tree_sem = ctx.enter_context(nc.semaphore("tree_sem"))
        cp_sem = ctx.enter_context(nc.semaphore("cp_sem"))
        st_sem = ctx.enter_context(nc.semaphore("st_sem"))
        s1st_sem = ctx.enter_context(nc.semaphore("s1st_sem"))
        warm_sem = ctx.enter_context(nc.semaphore("warm_sem"))

        block = ctx.enter_context(nc.Block(no_gpsimd_drain=True))

        @block.sync
        def _(sync):
            # Full input loads. The urgent prologue slice (kq0) goes on the
            # gpsimd engine, which enters its block ~1us earlier than sync.
            # Each barrier gets its own semaphore: a shared counter races
            # when DMA queues progress unevenly.
            sync.dma_start(out=kt_sb[:, 0, :], in_=kt_in[0]).then_inc(load_sems[1], 16)
            sync.dma_start(out=qt_sb[:, 0, :], in_=qt_in[0]).then_inc(load_sems[3], 16)
            sync.dma_start(out=kt_sb[:, 1, :], in_=kt_in[1]).then_inc(load_sems[4], 16)
            sync.dma_start(out=qt_sb[:, 1, :], in_=qt_in[1]).then_inc(load_sems[6], 16)
            # Output ctx DMAs from here (sync is idle after the loads);
            # the s1 DMAs stay on gpsimd so the triggers run in parallel.
            for hq in range(NHQ):
                h, qc = divmod(hq, NQC)
                sync.wait_ge(cp_sem, hq + 1)
                sync.dma_start(
                    out=ctx_out[h][:, qc * QC : (qc + 1) * QC],
                    in_=out_sb[:, hq % 2, :],
                ).then_inc(st_sem, 16)
            sync.wait_ge(st_sem, 16 * NHQ)

        @block.tensor
        def _(tensor):
            def emit_qk(g):
                h, r = divmod(g, NQC * NT)
                qc, kt = divmod(r, NT)
                b = g % NQKBUF
                if g == 0:
                    tensor.wait_ge(load_sems[0], 16)   # kq0 prologue slice
                if g == NQKBUF:
                    tensor.wait_ge(load_sems[1], 16)   # kt0 full
                    tensor.wait_ge(load_sems[3], 16)   # qt0 full
                if g == NQC * NT:
                    tensor.wait_ge(load_sems[4], 16)   # kt1
                    tensor.wait_ge(load_sems[6], 16)   # qt1
                if g < NQKBUF:
                    lhs = kq0_sb[:, kt * 128 : (kt + 1) * 128]
                    rhs = kq0_sb[:, 768:1280]
                else:
                    lhs = kt_sb[:, h, kt * 128 : (kt + 1) * 128]
                    rhs = qt_sb[:, h, qc * QC : (qc + 1) * QC]
                tensor.matmul(
                    psum_qk[:, b * QC : (b + 1) * QC],
                    lhs,
                    rhs,
                    start=True,
                    stop=True,
                ).then_inc(qk_sem, 1)

            def emit_pv(g2):
                h2, r2 = divmod(g2, NQC * NT)
                qc2, kt2 = divmod(r2, NT)
                hq2 = g2 // NT
                cb = hq2 % 2
                pb2 = hq2 % 3
                if kt2 == 0:
                    tensor.wait_ge(load_sems[2] if h2 == 0 else load_sems[5], 16)
                    if hq2 >= 2:
                        tensor.wait_ge(cp_sem, hq2 - 1)  # psum_cx buf drained
                mm = tensor.matmul(
                    psum_cx[:, cb * QC : (cb + 1) * QC],
                    va_sb[:, h2, kt2, :],
                    pt_sb[pb2][:, kt2, :],
                    start=(kt2 == 0),
                    stop=(kt2 == NT - 1),
                )
                if kt2 == NT - 1:
                    mm.then_inc(pv_sem, 1)

            for g in range(NQKBUF):
                emit_qk(g)
            for gi, (g0, n, hq, ls) in enumerate(GRPS):
                tensor.wait_ge(exp_sem, gi + 1)
                for g in range(g0 + NQKBUF, min(g0 + NQKBUF + n, NSTEP)):
                    emit_qk(g)
                for g2 in range(g0, g0 + n):
                    emit_pv(g2)

        @block.scalar
        def _(scalar):
            import concourse.mybir as mybir_

            # Warm the Exp activation table while input DMAs run.
            scalar.wait_ge(warm_sem, 1)
            scalar.activation(
                out=warm_sb[:, 1:2],
                in_=warm_sb[:, 0:1],
                func=mybir_.ActivationFunctionType.Exp,
                scale=1.0,
            )
            for gi, (g0, n, hq, ls) in enumerate(GRPS):
                if ls == 0 and hq >= 3:
                    # pt buf reused from (hq-3): both PV and tree must be done
                    scalar.wait_ge(pv_sem, hq - 2)
                    scalar.wait_ge(tree_sem, hq - 2)
                scalar.wait_ge(qk_sem, g0 + n)
                b0 = g0 % NQKBUF
                scalar.activation(
                    out=pt_sb[hq % 3][:, ls : ls + n, :],
                    in_=psum_qk[:, b0 * QC : (b0 + n) * QC],
                    func=mybir_.ActivationFunctionType.Exp,
                    scale=SCALE,
                ).then_inc(exp_sem, 1)

        @block.vector
        def _(vector):
            # incremental kt-accumulation: after each exp group, fold its
            # k-tiles into s1 (fp32 accumulator) so the denominator finishes
            # right after the chunk's last exp instead of in a tail tree.
            for gi, (g0, n, hq, ls) in enumerate(GRPS):
                pb = hq % 3
                sb1 = hq % 2
                first = ls == 0
                last = ls + n == NT
                vector.wait_ge(exp_sem, gi + 1)
                if first and hq >= 2:
                    vector.wait_ge(s1st_sem, 16 * (hq - 1))
                with nc.allow_low_precision("bf16 partials; fp32 accumulator"):
                    # reduce the group's n planes to one bf16 partial in t1
                    # (or directly into the fp32 accumulator when cheap)
                    acc = s1_sb[:, sb1, :]
                    if n == 4:
                        vector.tensor_add(
                            out=s2_sb[:, :, :],
                            in0=pt_sb[pb][:, ls : ls + 2, :],
                            in1=pt_sb[pb][:, ls + 2 : ls + 4, :],
                        )
                        if first:
                            mm = vector.tensor_add(out=acc, in0=s2_sb[:, 0, :], in1=s2_sb[:, 1, :])
                        else:
                            vector.tensor_add(out=t1_sb[:, :], in0=s2_sb[:, 0, :], in1=s2_sb[:, 1, :])
                            mm = vector.tensor_add(out=acc, in0=acc, in1=t1_sb[:, :])
                    elif n == 3:
                        vector.tensor_add(
                            out=t1_sb[:, :], in0=pt_sb[pb][:, ls, :], in1=pt_sb[pb][:, ls + 1, :]
                        )
                        if first:
                            mm = vector.tensor_add(out=acc, in0=t1_sb[:, :], in1=pt_sb[pb][:, ls + 2, :])
                        else:
                            vector.tensor_add(out=t1_sb[:, :], in0=t1_sb[:, :], in1=pt_sb[pb][:, ls + 2, :])
                            mm = vector.tensor_add(out=acc, in0=acc, in1=t1_sb[:, :])
                    elif n == 2:
                        if first:
                            mm = vector.tensor_add(
                                out=acc, in0=pt_sb[pb][:, ls, :], in1=pt_sb[pb][:, ls + 1, :]
                            )
                        else:
                            vector.tensor_add(
                                out=t1_sb[:, :], in0=pt_sb[pb][:, ls, :], in1=pt_sb[pb][:, ls + 1, :]
                            )
                            mm = vector.tensor_add(out=acc, in0=acc, in1=t1_sb[:, :])
                    else:  # n == 1
                        if first:
                            mm = vector.tensor_scalar_add(out=acc, in0=pt_sb[pb][:, ls, :], scalar1=0.0)
                        else:
                            mm = vector.tensor_add(out=acc, in0=acc, in1=pt_sb[pb][:, ls, :])
                if last:
                    mm.then_inc(tree_sem, 1)
                    # ctxT PSUM -> SBUF staging copy
                    vector.wait_ge(pv_sem, hq + 1)
                    if hq >= 2:
                        vector.wait_ge(st_sem, 16 * (hq - 1))
                    cb = hq % 2
                    vector.tensor_scalar_add(
                        out=out_sb[:, cb, :],
                        in0=psum_cx[:, cb * QC : (cb + 1) * QC],
                        scalar1=0.0,
                    ).then_inc(cp_sem, 1)

        @block.gpsimd
        def _(gpsimd):
            # Most urgent first: the prologue Q/K slice that QK(0..5) needs.
            gpsimd.dma_start(out=kq0_sb[:, :], in_=kq0_in[:, :]).then_inc(load_sems[0], 16)
            gpsimd.memset(warm_sb[:, 0:1], 0.0).then_inc(warm_sem, 1)
            gpsimd.dma_start(out=va_sb[:, 0, :, :], in_=va_in[0]).then_inc(load_sems[2], 16)
            gpsimd.dma_start(out=va_sb[:, 1, :, :], in_=va_in[1]).then_inc(load_sems[5], 16)
            for hq in range(NHQ):
                h, qc = divmod(hq, NQC)
                gpsimd.wait_ge(tree_sem, hq + 1)
                gpsimd.dma_start(
                    out=s_out[h, qc], in_=s1_sb[:, hq % 2, :]
                ).then_inc(s1st_sem, 16)
            gpsimd.wait_ge(s1st_sem, 16 * NHQ)

    return nc


def _perm_blocks(x, idx):
    xb = x.reshape(B, S // BLOCK, BLOCK, H, D)
    return xb[:, idx].reshape(B, S, H, D)


def kernel(**inputs):
    from concourse.bass_utils import run_bass_kernel_spmd

    q = np.asarray(inputs["query"], dtype=np.float32)
    k = np.asarray(inputs["key"], dtype=np.float32)
    v = np.asarray(inputs["value"], dtype=np.float32)
    hp = np.asarray(inputs["head_perm_idx"]).astype(np.int64)
    hd = np.asarray(inputs["head_deperm_idx"]).astype(np.int64)
    rp = np.asarray(inputs["new_row_perm_idx"]).astype(np.int64)
    cp = np.asarray(inputs["new_col_perm_idx"]).astype(np.int64)
    rd = np.asarray(inputs["new_row_deperm_idx"]).astype(np.int64)

    qp = _perm_blocks(q[:, :, hp], rp)[0]  # [S, H, D]
    kp = _perm_blocks(k[:, :, hp], cp)[0]
    vp = _perm_blocks(v[:, :, hp], cp)[0]

    bf = ml_dtypes.bfloat16
    qt = np.ascontiguousarray(qp.transpose(1, 2, 0)).astype(bf)  # [H, D, S]
    kt = np.ascontiguousarray(kp.transpose(1, 2, 0)).astype(bf)  # [H, D, S]
    # va[h, kp, kt, d] = V[h, kt*128 + kp, d]
    va = np.ascontiguousarray(
        vp.transpose(1, 0, 2).reshape(H, NT, 128, D).transpose(0, 2, 1, 3)
    ).astype(bf)

    if "nc" not in _CACHE:
        _CACHE["nc"] = _build_bass()
    nc = _CACHE["nc"]

    core_ids = list(range(NCORES))
    in_maps = [
        {
            "kt_in": np.ascontiguousarray(kt[c * HPC : (c + 1) * HPC]),
            "qt_in": np.ascontiguousarray(qt[c * HPC : (c + 1) * HPC]),
            "va_in": np.ascontiguousarray(va[c * HPC : (c + 1) * HPC]),
            "kq0_in": np.ascontiguousarray(
                np.concatenate(
                    [kt[c * HPC][:, 0:768], qt[c * HPC][:, 0:QC]], axis=1
                )
            ),
        }
        for c in core_ids
    ]
    res = run_bass_kernel_spmd(nc, in_maps, core_ids)
    _CACHE["last_result"] = res

    ctxT = np.concatenate(
        [res.results[c]["ctx_out"] for c in core_ids], axis=0
    )  # [H, D, S] fp32, unnormalized
    s1 = np.concatenate(
        [np.asarray(res.results[c]["s_out"], dtype=np.float32) for c in core_ids],
        axis=0,
    )  # [H, NQC, 128, QC]
    denom = s1.sum(axis=2).reshape(H, S)  # [H, S]
    ctxT = ctxT / denom[:, None, :]
    ctx = np.ascontiguousarray(ctxT.transpose(2, 0, 1))[None]  # [1, S, H, D]
    ctx = _perm_blocks(ctx, rd)
    out = ctx[:, :, hd]
    return np.ascontiguousarray(out, dtype=np.float32)
